# revision 3
# baseline (speedup 1.0000x reference)
"""EmbeddingBag-group kernel for Trainium2 (8 NeuronCores, SPMD data-parallel).

Problem: 3 embedding tables [10M, 3] f32, shared flat indices [819200] in bags
of 50 (offsets = arange(16384)*50), mode='sum'; outputs tiled 5/10/6x and
concatenated -> [21*16384, 3].

Strategy:
  * Host fuses the 3 tables into one [10M+1, 9] f32 table (the three tables
    share the index vector -> one 36B-row gather per index instead of three).
    Row 10M is zeros = padding target for ragged bags.
  * Bags are data-parallel sharded across 8 cores (2048 bags/core), and across
    the 128 partitions within a core (16 blocks of 128 bags). The SWDGE
    indirect DMA consumes exactly one index per partition per command (HW
    ucode limit, verified by probing), so bag position j of 128 bags is one
    gather command writing a [128, 9] f32 slice: 800 commands/core at ~825ns
    of Pool-engine descriptor-generation each.
  * All 800 index columns are preloaded into SBUF with a single DMA; gather
    commands round-robin over 4 SWDGE queues; gather tiles triple-buffered so
    DVE reduction and writeback overlap the Pool-engine descriptor stream.
  * DVE segment-reduce sums the 50 positions -> [128, 9] per block; results
    DMA back to DRAM; host reassembles and applies the tile/concat epilogue.
"""

import numpy as np

N_CORES = 8
P = 128                  # SBUF partitions
V = 10_000_000           # rows per embedding table
D = 3                    # embedding dim per table
DF = 3 * D               # fused dim
B = 16384
L = 50

NQ = 4                   # SWDGE queues to spread gather commands over
GAT_BUFS = 4
OUT_BUFS = 2

_NC_CACHE: dict = {}


def _indirect_gather(eng, out_ap, table_ap, offset_ap, queue_name):
    """indirect_dma_start with a selectable SWDGE queue."""
    import concourse.mybir as mybir

    out_l = eng.lower_ap_dma(out_ap, for_indirect_dma=True)
    in_l = eng.lower_ap_dma(table_ap, for_indirect_dma=True)
    assert len(in_l) == 1 and len(out_l) == 1
    off_l = eng.lower_ap_dma(offset_ap)
    assert len(off_l) == 1
    in_l.append(off_l[0])
    ap_shape = table_ap.shape
    coef = 1
    for i in range(1, len(ap_shape)):
        coef *= ap_shape[i]
    dyn = mybir.DynamicAccessPatternInfo(
        c=0,
        actual_ap=out_ap.ap,
        indirect_dim_max_index=ap_shape[0],
        offset_expr=[
            mybir.DynamicAccessPatternOffsetExpr(
                coef=coef,
                aff_expr=mybir.DynamicAccessPatternOffsetExprAffExpr(
                    kind="IndirectArgId", arg_id=1
                ),
            )
        ],
    )
    in_l[0].dynamic_ap_info = dyn
    return eng.add_instruction(
        mybir.InstDMACopy(
            name=eng.bass.get_next_instruction_name(),
            queue=queue_name,
            mode="Copy",
            ins=in_l,
            outs=out_l,
            oob_is_err=True,
            cce_op=mybir.AluOpType.bypass,
        )
    )


def _build_nc(nblk: int, lp: int, v_rows: int, nq: int = NQ,
              repeat: int = 1, gat_bufs: int = GAT_BUFS):
    """Per-core Bass/Tile program: nblk blocks of 128 bags, lp idx/bag.

    idxs layout: [P, nblk*lp] int32 -- column g*lp+j holds the j-th index of
    the 128 bags of block g (one per partition). Loaded once.
    """
    import concourse.bacc as bacc
    import concourse.mybir as mybir
    import concourse.tile as tile

    nc = bacc.Bacc(
        "TRN2",
        target_bir_lowering=False,
        debug=False,
        num_devices=N_CORES,
        num_swdge_queues=nq,
    )

    table = nc.dram_tensor("table", [v_rows, DF], mybir.dt.float32, kind="ExternalInput")
    idxs = nc.dram_tensor("idxs", [P, nblk * lp], mybir.dt.int32, kind="ExternalInput")
    out = nc.dram_tensor("out", [nblk, P, DF], mybir.dt.float32, kind="ExternalOutput")

    qname = lambda i: f"qPoolDynamic{i % nq if i % nq else ''}"
    cmd = 0

    with tile.TileContext(nc) as tc:
        with (
            tc.tile_pool(name="idxp", bufs=1) as idxp,
            tc.tile_pool(name="gatp", bufs=gat_bufs) as gatp,
            tc.tile_pool(name="outp", bufs=OUT_BUFS) as outp,
        ):
            it = idxp.tile([P, nblk * lp], mybir.dt.int32)
            nc.sync.dma_start(out=it[:], in_=idxs[:])

            for _rep in range(repeat):
                for g in range(nblk):
                    gt = gatp.tile([P, lp * DF], mybir.dt.float32)
                    for j in range(lp):
                        _indirect_gather(
                            nc.gpsimd,
                            gt[:, j * DF:(j + 1) * DF],
                            table[:],
                            it[:, g * lp + j:g * lp + j + 1],
                            qname(cmd),
                        )
                        cmd += 1

                    ot = outp.tile([P, DF], mybir.dt.float32)
                    rin = gt[:].rearrange("p (l k) -> p k l", l=lp, k=DF)
                    rout = ot[:].rearrange("p k -> p k")
                    nc.vector.reduce_sum(out=rout, in_=rin, axis=mybir.AxisListType.X)

                    nc.sync.dma_start(out=out[g], in_=ot[:])

    nc.compile()
    return nc


def _get_nc(nblk, lp, v_rows):
    key = (nblk, lp, v_rows, NQ)
    if key not in _NC_CACHE:
        _NC_CACHE[key] = _build_nc(nblk, lp, v_rows)
    return _NC_CACHE[key]


def _bag_matrix(idx: np.ndarray, off: np.ndarray, n: int, b: int, pad_row: int):
    """[B, LP] int32 matrix of per-bag indices, padded with pad_row."""
    lengths = np.diff(off, append=np.int64(n))
    if off[0] == 0 and np.all(lengths == lengths[0]):
        return idx.reshape(b, int(lengths[0])), int(lengths[0])
    lp = int(lengths.max())
    mat = np.full((b, lp), pad_row, np.int32)
    pos = np.arange(n, dtype=np.int64)
    seg = np.searchsorted(off, pos, side="right") - 1
    valid = seg >= 0
    within = pos[valid] - off[seg[valid]]
    mat[seg[valid], within] = idx[valid]
    return mat, lp


def _prepare(eb_input, eb_offset, table0, table1, table2):
    idx = np.asarray(eb_input).astype(np.int32, copy=False)
    off = np.asarray(eb_offset).astype(np.int64, copy=False)
    n, b = idx.shape[0], off.shape[0]

    t0 = np.asarray(table0, dtype=np.float32)
    v = t0.shape[0]
    T = np.zeros((v + 1, DF), np.float32)
    T[:v, 0:D] = t0
    T[:v, D:2 * D] = np.asarray(table1, dtype=np.float32)
    T[:v, 2 * D:3 * D] = np.asarray(table2, dtype=np.float32)

    bag_idx, lp = _bag_matrix(idx, off, n, b, pad_row=v)
    bag_idx = np.ascontiguousarray(bag_idx, dtype=np.int32)

    bags_per_core = b // N_CORES
    assert bags_per_core % P == 0
    nblk = bags_per_core // P

    in_maps = []
    for c in range(N_CORES):
        a = bag_idx[c * bags_per_core:(c + 1) * bags_per_core].reshape(nblk, P, lp)
        # idxs[p, g*lp+j] = bag_idx[block g, partition p, j]
        a = a.transpose(1, 0, 2).reshape(P, nblk * lp)
        in_maps.append({"table": T, "idxs": np.ascontiguousarray(a)})
    return in_maps, nblk, lp, v, bags_per_core


def _assemble(results, nblk, bags_per_core):
    parts = []
    for c in range(N_CORES):
        o = results[c]["out"]              # [nblk, P, DF]
        parts.append(o.reshape(bags_per_core, DF))
    full = np.concatenate(parts, axis=0)   # [B, 9]
    p0 = full[:, 0:D]
    p1 = full[:, D:2 * D]
    p2 = full[:, 2 * D:3 * D]
    return np.concatenate(
        [np.tile(p0, (5, 1)), np.tile(p1, (10, 1)), np.tile(p2, (6, 1))], axis=0
    )


def kernel(eb_input, eb_offset, table0, table1, table2) -> np.ndarray:
    from concourse.bass_utils import run_bass_kernel_spmd

    in_maps, nblk, lp, v, bags_per_core = _prepare(
        eb_input, eb_offset, table0, table1, table2
    )
    nc = _get_nc(nblk, lp, v + 1)
    res = run_bass_kernel_spmd(nc, in_maps, core_ids=list(range(N_CORES)))
    return _assemble(res.results, nblk, bags_per_core)
